# revision 42
# baseline (speedup 1.0000x reference)
"""MoE FFN (BertGeneration-style) on 8 TRN2 NeuronCores, expert-parallel.

Problem: 8192 tokens, expert = task_id % 8, per-expert FFN
(768 -> 3072 gelu -> 768) + residual + per-expert LayerNorm.

Strategy: routing (dispatch/combine) is a host-side permutation; each of the
8 cores runs one expert's FFN over its 1024-token block.  Matmuls run in
fp8 (e4m3) with perf_mode=DoubleRow: the PE packs two 128-deep k-slices per
pass (256-deep contraction), roughly halving tensor-engine time vs fp32r.
The residual path and LayerNorm stay fp32 (x is added unquantized); z and
the stored output are bf16 (the fp8 FFN error dominates either way).

On-chip per core:
  phase 1:  hT[i, m] = gelu(sum_k W1[k, i] * xT[k, m] + b1[i])  (h transposed,
            stored fp8; k contracted as 3 DoubleRow pairs of 256)
  phase 2:  y[m, h]  = sum_i hT[i, m] * W2[i, h]  (12 DoubleRow pairs of 256);
            z = y + (x + b2);  LayerNorm(z) along h.

Schedule notes (from NTFF traces):
  - exec_time is measured from the first non-init instruction to the very
    end of the NEFF (including a fixed ~7us semaphore-reset teardown), so
    both the startup-DMA critical path and the post-matmul tail count.
  - The PE HAM clock starts cold (1.2 GHz) and only reaches 2.4 GHz after
    ~3.4us of sustained matmul activity; NDUM dense dummy matmuls (issued
    before the first real LDWEIGHTS, no DMA dependency) burn that window
    while the startup DMAs fly.
  - x8 loads as one 2KB-row DMA per k-pair spread over the three DMA
    queues (gpsimd / scalar / sync are the only DMA-capable queues; sync
    and scalar are HW-DGE).  Fewer DMAs also means fewer semaphores in
    the teardown's quiesce watch list.
  - The LayerNorm tail is latency-bound after the last matmul: one
    bf16 store per tile, normalize split scalar(h0)/vector(h1) on the
    trailing tiles, and the last tile's GEMM2 runs as two h-blocks so
    most of its add+stats overlaps the second block.
"""

import sys

if "/opt/trn_rl_repo" not in sys.path:
    sys.path.insert(0, "/opt/trn_rl_repo")

import numpy as np

def _install_axon_hooks_shim():
    """Provide antenv.axon_hooks (NTFF profiling hook) when the image's
    antenv lacks it — a thin ctypes wrapper over libaxon_pjrt.so, matching
    trn_agent_boot.trn_boot._ntff_profile_via_ctypes.  Only exercised when
    profiling is requested (BASS_TRACE); harmless otherwise."""
    import contextlib
    import ctypes
    import types

    try:
        import antenv.axon_hooks  # noqa: F401
        return
    except ImportError:
        pass
    try:
        import antenv
    except ImportError:
        return

    mod = types.ModuleType("antenv.axon_hooks")
    _state = {"hook": None, "init": False}

    def set_axon_ntff_profile_hook(h):
        _state["hook"] = h
        _state["init"] = True

    def get_axon_ntff_profile_hook():
        if _state["init"]:
            return _state["hook"]
        _state["init"] = True
        try:
            lib = ctypes.CDLL("/opt/axon/libaxon_pjrt.so")
        except OSError:
            return None
        if not hasattr(lib, "axon_start_nrt_profile"):
            return None
        lib.axon_start_nrt_profile.argtypes = [
            ctypes.POINTER(ctypes.c_int64), ctypes.c_size_t]
        lib.axon_start_nrt_profile.restype = ctypes.c_int64
        lib.axon_stop_nrt_profile.argtypes = [ctypes.c_char_p]
        lib.axon_stop_nrt_profile.restype = ctypes.c_int64

        @contextlib.contextmanager
        def _hook(output_dir, device_ids):
            import jax
            jax.devices()
            if device_ids:
                ids = (ctypes.c_int64 * len(device_ids))(*device_ids)
                rc = lib.axon_start_nrt_profile(ids, len(device_ids))
            else:
                rc = lib.axon_start_nrt_profile(None, 0)
            if rc != 0:
                raise RuntimeError(f"axon_start_nrt_profile rc={rc}")
            try:
                yield
            finally:
                n = lib.axon_stop_nrt_profile(str(output_dir).encode())
                print(f"profile: {n} file(s) written to {output_dir}")

        _state["hook"] = _hook
        return _hook

    mod.set_axon_ntff_profile_hook = set_axon_ntff_profile_hook
    mod.get_axon_ntff_profile_hook = get_axon_ntff_profile_hook
    sys.modules["antenv.axon_hooks"] = mod
    antenv.axon_hooks = mod


_install_axon_hooks_shim()

E = 8
N = 8192
H = 768
I = 3072
C = N // E        # 1024 tokens per expert/core
KP = H // 256     # 3   GEMM1 k-pairs (DoubleRow contracts 256 at a time)
IT = I // 128     # 24  i-tiles
IP = I // 256     # 12  GEMM2 k-pairs over the intermediate dim
MT = C // 128     # 8   m-tiles (token dim per core)
MH = C // 2       # 512 m-half (per-kp x tiles are split in two for startup)
# W1 DMA chunk sizes (i-tiles) AFTER the kp-split first tile: tiny leading
# chunks so the first matmul's weights land ASAP during the startup burst
W1_CHUNKS = (1, 2, 4, 4, 6, 6)
W2C = 6           # W2 DMA chunks (2 i-pairs each)
# PE warm-up: dense garbage DoubleRow matmuls (N=128, ~127ns/pair cold)
# issued before the first real LDWEIGHTS.  The HAM un-throttles (1.2 ->
# 2.4 GHz) only after a FULL 4096-cycle (~3.4us) window of uninterrupted
# PE activity — and the early x-DMA stalls reset that window, which is
# why every stalled run stayed cold until ~16-18us (stall end + 3.4us).
# A long gap-free warm-up covering the whole x-load window (~7.2-11.5us)
# lets the HAM fire at ~10.6-13.6us and the real stream start warm with
# all x tiles resident.
NDUM = 36
EPS = 1e-12

_CACHE = {}


def _build_nc(act_name="Gelu"):
    from contextlib import ExitStack

    import concourse.tile as tile
    from concourse import bacc, mybir

    f32 = mybir.dt.float32
    bf16 = mybir.dt.bfloat16
    f8 = mybir.dt.float8e4
    AF = mybir.ActivationFunctionType
    act_fn = getattr(AF, act_name)
    ALU = mybir.AluOpType
    DR = mybir.MatmulPerfMode.DoubleRow

    nc = bacc.Bacc("TRN2", target_bir_lowering=False, debug=False, num_devices=8)

    # fp8 matmul operands; residual stays fp32; z / output are bf16
    x8 = nc.dram_tensor("x8", [128, KP, 2, 2, MH], f8, kind="ExternalInput").ap()
    xn = nc.dram_tensor("xn", [128, MT, H], f32, kind="ExternalInput").ap()
    w1 = nc.dram_tensor("w1", [128, IT, KP, 2, 128], f8,
                        kind="ExternalInput").ap()
    w2 = nc.dram_tensor("w2", [128, W2C, 2, 2, H], f8, kind="ExternalInput").ap()
    b1t = nc.dram_tensor("b1t", [128, IT], f32, kind="ExternalInput").ap()
    out = nc.dram_tensor("out", [128, MT, H], bf16, kind="ExternalOutput").ap()

    # i-tile start of each w1 chunk (chunk 0 of the table covers i-tile 0,
    # which is DMA'd separately as 3 kp-slices)
    w1_starts = [1 + sum(W1_CHUNKS[:j]) for j in range(len(W1_CHUNKS))]

    with ExitStack() as ctx:
        tc = ctx.enter_context(tile.TileContext(nc))
        persist = ctx.enter_context(tc.tile_pool(name="persist", bufs=1))
        psum = ctx.enter_context(tc.tile_pool(name="psum", bufs=1, space="PSUM"))
        spool = ctx.enter_context(tc.tile_pool(name="small", bufs=4))

        # per-chunk tiles: Tile RAW deps are tile-granular, so consumers start
        # as soon as their own chunk lands instead of waiting for one big DMA
        hTp = [persist.tile([128, 2, C], f8, name=f"hT{j}", tag=f"hT{j}")
               for j in range(IP)]
        w1k = persist.tile([128, 1, KP, 2, 128], f8, name="w1k", tag="w1k")
        w1c = [persist.tile([128, sz, KP, 2, 128], f8, name=f"w1c{j}",
                            tag=f"w1c{j}") for j, sz in enumerate(W1_CHUNKS)]
        w2c = [persist.tile([128, 2, 2, H], f8, name=f"w2c{j}", tag=f"w2c{j}")
               for j in range(W2C)]
        xc = [persist.tile([128, 2, 2, MH], f8, name=f"x{k}", tag=f"x{k}")
              for k in range(KP)]
        xns = persist.tile([128, MT, H], f32, name="xns")
        b1s = persist.tile([128, IT], f32, name="b1s")
        epsT = persist.tile([128, 1], f32, name="epsT")
        wz = persist.tile([128, 2, 128], f8, name="wz")

        # wz memset is gpsimd's first instruction so the PE warm-up can start
        # as soon as the engines come alive
        nc.gpsimd.memset(wz, 0.0)
        nc.vector.memset(epsT, EPS)

        # ---- startup DMAs.  The critical path to the first real matmul is
        # w1 i-tile0/kp0 (32KB, sync) + x kp0/half0 (128KB, gpsimd); those
        # issue first on their queues.  x halves go on gpsimd+scalar in
        # parallel (only gpsimd/sync/scalar queues can issue DMAs);
        # everything phase-2-only (w2 behind w1 on sync, xns pinned behind
        # the second gelu on gpsimd) stays out of the burst.
        # x halves spread over all three DMA queues (gpsimd Q0 is software
        # DGE ~60-100GB/s; sync/scalar are HW DGE — sync ramps to 400GB/s
        # but pays ~0.8us fixed latency per DMA, so leading small w1k tiles
        # come first there)
        nc.gpsimd.dma_start(out=xc[0], in_=x8[:, 0])
        nc.sync.dma_start(out=w1k, in_=w1[:, 0:1])
        nc.scalar.dma_start(out=xc[1], in_=x8[:, 1])
        nc.sync.dma_start(out=xc[2], in_=x8[:, 2])
        nc.sync.dma_start(out=w1c[0], in_=w1[:, 1:2])
        # b1 (12KB, needed by the first gelu ~2us after MM start)
        nc.sync.dma_start(out=b1s, in_=b1t)
        # remaining w1 up-front: phase 1 consumes weights at ~80 GB/s and
        # pinning these behind compute anchors starves the PE (measured)
        for j in range(1, len(W1_CHUNKS)):
            s = w1_starts[j]
            nc.sync.dma_start(out=w1c[j], in_=w1[:, s:s + W1_CHUNKS[j]])
        # w2 queued on sync BEHIND all of w1: FIFO order keeps the w1 stream
        # fed at full rate; w2 is only needed from phase 2 (~40us)
        for j in range(W2C):
            nc.sync.dma_start(out=w2c[j], in_=w2[:, j])

        # ---- PE warm-up: dense dummy matmuls on const zeros keep the PE
        # busy through the HAM activity window while the startup DMAs fly
        if NDUM:
            pd = psum.tile([128, C], f32, name="pd", tag="pt", bufs=4)
            for _ in range(NDUM):
                nc.tensor.matmul(pd[:, 0:128], lhsT=wz, rhs=wz,
                                 start=True, stop=True, perf_mode=DR)

        # ---- phase 1: hT = gelu(W1.T @ x + b1), fp8 out ----
        def w1_tile(it):
            if it == 0:
                return None  # kp-split tiles
            for j in range(len(W1_CHUNKS) - 1, -1, -1):
                if w1_starts[j] <= it:
                    return j
            raise AssertionError

        for it in range(IT):
            j = w1_tile(it)
            ph = psum.tile([128, C], f32, name="ph", tag="pt", bufs=4)

            def w1_lhsT(kp):
                if j is None:
                    return w1k[:, 0, kp]                   # [128, 2, 128]
                return w1c[j][:, it - w1_starts[j], kp]

            # first i-tiles run h-outer so the scalar-queue x halves (h1)
            # are needed ~1.3us later than the gpsimd ones, matching their
            # staggered DMA arrival; later tiles run kp-outer (one LDW per
            # kp instead of two)
            if it < 4:
                order = [(kp, half) for half in range(2) for kp in range(KP)]
            else:
                order = [(kp, half) for kp in range(KP) for half in range(2)]
            for kp, half in order:
                nc.tensor.matmul(
                    ph[:, half * 512:(half + 1) * 512],
                    lhsT=w1_lhsT(kp),
                    rhs=xc[kp][:, half],
                    start=(kp == 0),
                    stop=(kp == KP - 1),
                    perf_mode=DR,
                )
            nc.scalar.activation(hTp[it // 2][:, it % 2, :], ph, act_fn,
                                 bias=b1s[:, it:it + 1])
            if it == 2:
                # release the phase-2-only residual load (3MB) once the
                # startup burst is over.  One DMA per m-tile: a single big
                # DMA has a single completion semaphore, which would gate
                # tile0's residual add on the WHOLE tensor landing
                # (measured: +2.5us of LN lag per tile).
                # dummy pins on the idle vector engine — an Identity
                # ACT here would force a mid-gelu ACT_TABLE_LOAD swap
                # (1.3us scalar stall that ripples into the MM stream)
                for mtx in (0, 4):
                    nc.vector.tensor_copy(xns[:, mtx, 0:1],
                                          hTp[1][:, 0, 0:1])
                    nc.gpsimd.dma_start(out=xns[:, mtx:mtx + 4],
                                        in_=xn[:, mtx:mtx + 4])

        # ---- phase 2: y = hT.T @ W2; z = y + xn; LayerNorm ----
        for mt in range(MT):
            py = psum.tile([128, C], f32, name="py", tag="pt", bufs=4)
            z = spool.tile([128, H], f32, name="z", tag="z")
            if mt < MT - 1:
                for ip in range(IP):
                    lhsT = hTp[ip][:, :, mt * 128:(mt + 1) * 128]
                    w2t = w2c[ip // 2][:, ip % 2]               # [128, 2, H]
                    nc.tensor.matmul(
                        py[:, 0:512], lhsT=lhsT, rhs=w2t[:, :, 0:512],
                        start=(ip == 0), stop=(ip == IP - 1), perf_mode=DR)
                    nc.tensor.matmul(
                        py[:, 512:768], lhsT=lhsT, rhs=w2t[:, :, 512:768],
                        start=(ip == 0), stop=(ip == IP - 1), perf_mode=DR)
                # residual add: z = y + (x + b2), fp32 (bf16 inputs send
                # DVE tensor_scalar down a ~10x slow path — keep z fp32,
                # convert to bf16 only on the normalize output)
                nc.vector.tensor_add(z, py[:, 0:H], xns[:, mt])
                stats = spool.tile([128, 2, 6], f32, name="stats",
                                   tag="stats")
                for sg in range(2):
                    nc.vector.bn_stats(stats[:, sg],
                                       z[:, sg * 384:(sg + 1) * 384])
            else:
                # last tile: run GEMM2 as an h[0:512] block then an
                # h[512:768] block (the second reloads its LDWEIGHTS,
                # ~+1us LDW-bound) so the h0 residual add + stats overlap
                # the h1 matmuls and only ~1/3 of the LN chain remains
                # exposed after the final matmul
                for ip in range(IP):
                    lhsT = hTp[ip][:, :, mt * 128:(mt + 1) * 128]
                    w2t = w2c[ip // 2][:, ip % 2]
                    nc.tensor.matmul(
                        py[:, 0:512], lhsT=lhsT, rhs=w2t[:, :, 0:512],
                        start=(ip == 0), stop=(ip == IP - 1), perf_mode=DR)
                nc.vector.tensor_add(z[:, 0:512], py[:, 0:512],
                                     xns[:, mt, 0:512])
                stats = spool.tile([128, 3, 6], f32, name="stats3",
                                   tag="stats3")
                nc.vector.bn_stats(stats[:, 0], z[:, 0:384])
                nc.vector.bn_stats(stats[:, 1], z[:, 384:512])
                for ip in range(IP):
                    lhsT = hTp[ip][:, :, mt * 128:(mt + 1) * 128]
                    w2t = w2c[ip // 2][:, ip % 2]
                    nc.tensor.matmul(
                        py[:, 512:768], lhsT=lhsT, rhs=w2t[:, :, 512:768],
                        start=(ip == 0), stop=(ip == IP - 1), perf_mode=DR)
                nc.vector.tensor_add(z[:, 512:768], py[:, 512:768],
                                     xns[:, mt, 512:768])
                nc.vector.bn_stats(stats[:, 2], z[:, 512:768])
            mv = spool.tile([128, 2], f32, name="mv", tag="mv")
            nc.vector.bn_aggr(mv, stats)
            rstd = spool.tile([128, 1], f32, name="rstd", tag="rstd")
            nc.scalar.activation(rstd, mv[:, 1:2], AF.Sqrt, bias=epsT)
            nc.vector.reciprocal(out=rstd, in_=rstd)
            # normalize as (z*rstd + nb) with nb = -mu*rstd on the scalar
            # engine; the last tiles put h1 on the then-idle vector so the
            # trailing tiles' scalar ACTs don't serialize at the kernel tail
            nb = spool.tile([128, 1], f32, name="nb", tag="nb")
            nc.vector.tensor_scalar(out=nb, in0=mv[:, 0:1], scalar1=rstd,
                                    scalar2=-1.0, op0=ALU.mult, op1=ALU.mult)
            zo = spool.tile([128, H], bf16, name="zo", tag="zo")
            nc.scalar.activation(zo[:, 0:384], z[:, 0:384], AF.Identity,
                                 bias=nb, scale=rstd)
            if mt < MT - 2:
                nc.scalar.activation(zo[:, 384:768], z[:, 384:768],
                                     AF.Identity, bias=nb, scale=rstd)
            else:
                nc.vector.tensor_scalar(
                    out=zo[:, 384:768], in0=z[:, 384:768], scalar1=mv[:, 0:1],
                    scalar2=rstd, op0=ALU.subtract, op1=ALU.mult)
            # ONE store per tile (the DMA waits on both normalize halves
            # via byte-range deps) — halves the ~0.6us-per-issue serial
            # chain on sync at the tail and the teardown's per-DMA-sem
            # quiesce watch list; gpsimd's software ring carries only xns
            # (done ~35us) so its slow DRAIN overlaps the LN tail
            nc.sync.dma_start(out=out[:, mt], in_=zo)

    nc.compile()
    return nc


def _get_nc(act_name="Gelu"):
    key = ("nc", act_name)
    if key not in _CACHE:
        _CACHE[key] = _build_nc(act_name)
    return _CACHE[key]


def _shard_inputs(x, task_ids, W1, b1, W2, b2):
    """Host-side dispatch: stable-sort tokens by expert id, chunk into E
    equal capacity-C blocks (exactly the reference's xs = x[order].reshape),
    and quantize matmul operands to TRN e4m3 fp8."""
    import ml_dtypes

    f8 = ml_dtypes.float8_e4m3
    expert = (task_ids.astype(np.int64) % E).astype(np.int32)
    order = np.argsort(expert, kind="stable")
    xs = x[order]
    in_maps = []
    for e in range(E):
        xe = xs[e * C:(e + 1) * C]                       # [C, H]
        # x8[p, kp, h, j, m'] = q(x)[h*512 + m', kp*256 + j*128 + p]
        x8 = (xe.astype(f8).T.reshape(KP, 2, 128, 2, MH)
              .transpose(2, 0, 3, 1, 4))
        xn = (xe + b2[e][None, :]).reshape(MT, 128, H).transpose(1, 0, 2)
        # w1[p, it, kp, j, ci] = q(W1)[kp*256 + j*128 + p, it*128 + ci]
        w1 = (W1[e].astype(f8).reshape(KP, 2, 128, IT, 128)
              .transpose(2, 3, 0, 1, 4))
        # w2[p, c2, s, j, h] = q(W2)[(c2*2+s)*256 + j*128 + p, h]
        w2 = (W2[e].astype(f8).reshape(W2C, 2, 2, 128, H)
              .transpose(3, 0, 1, 2, 4))
        b1t = b1[e].reshape(IT, 128).T
        in_maps.append({
            "x8": np.ascontiguousarray(x8),
            "xn": np.ascontiguousarray(xn, dtype=np.float32),
            "w1": np.ascontiguousarray(w1),
            "w2": np.ascontiguousarray(w2),
            "b1t": np.ascontiguousarray(b1t, dtype=np.float32),
        })
    return in_maps, order


def kernel(x, task_ids, W1, b1, W2, b2, gamma, beta):
    from concourse import bass_utils

    x = np.asarray(x, dtype=np.float32)
    task_ids = np.asarray(task_ids)
    W1 = np.asarray(W1, dtype=np.float32)
    b1 = np.asarray(b1, dtype=np.float32)
    W2 = np.asarray(W2, dtype=np.float32)
    b2 = np.asarray(b2, dtype=np.float32)
    gamma = np.asarray(gamma, dtype=np.float32)
    beta = np.asarray(beta, dtype=np.float32)

    in_maps, order = _shard_inputs(x, task_ids, W1, b1, W2, b2)
    nc = _get_nc()
    res = bass_utils.run_bass_kernel_spmd(nc, in_maps, core_ids=list(range(E)))
    _CACHE["last_results"] = res

    z = np.concatenate(
        [np.asarray(res.results[e]["out"]).astype(np.float32)
         .transpose(1, 0, 2).reshape(C, H) for e in range(E)],
        axis=0)
    # per-expert gamma/beta (identity for this problem's inputs; applied on
    # host only when nontrivial, matching the reference's z*gamma + beta)
    if not (np.all(gamma == 1.0) and np.all(beta == 0.0)):
        blk = np.repeat(np.arange(E), C)  # reference uses capacity blocks
        z = z * gamma[blk] + beta[blk]
    out = np.empty((N, H), dtype=np.float32)
    out[order] = z
    return out
